# revision 11
# baseline (speedup 1.0000x reference)
"""Trainium2 Bass kernel for nn_AugmentedTransformer (encoder-decoder transformer).

Sharding: data-parallel over batch. B=32 sequences -> 4 per core x 8 cores.
Each core runs the full 6+6-layer model on its 4 sequences (1024 tokens).
Weights are bf16-cast host-side and replicated to every core; all matmuls run
in bf16 with fp32 PSUM accumulation; layernorm / softmax statistics in fp32.

Host-side prep (inside kernel()):
  - embedding gathers + positional encoding -> dense f32 activations
  - LN gammas, the 1/sqrt(64) attention scale, and the encoder-final-norm
    gamma are folded into the adjacent weight matrices (exact folds).
  - LN beta / conv bias c2b / gen_b are zeros in this problem's setup and the
    LN denominators' beta is zero too; c1b is applied exactly via the Relu
    activation per-partition bias. src/tgt masks are all-ones by spec
    (fill: ones); the decoder causal mask is applied as a static additive
    mask before the exp.

Device schedule: activations stay resident in SBUF. Attention computes S
transposed (softmax denominator fused into the O matmul via a ones-column in
V), so no probability transposes are needed. All phases are half-pipelined
over token tiles so the PE never stalls on LN statistics (which are computed
during the previous block's matmuls) -- this keeps the PE HAM clock at 2.4GHz.
"""

import math
import numpy as np
import ml_dtypes

import concourse.bass as bass
import concourse.tile as tile
import concourse.mybir as mybir
from concourse import bacc
import concourse.bass_utils as bass_utils

dt = mybir.dt
F32 = dt.float32
BF16 = dt.bfloat16
AF = mybir.ActivationFunctionType
ALU = mybir.AluOpType
NPBF16 = ml_dtypes.bfloat16

B, L, E, KD, H, HD, NB, VT = 32, 256, 512, 64, 8, 2048, 6, 66
NCORES = 8
BL = B // NCORES          # 4 sequences per core
T = BL * L                # 1024 tokens per core
NT = T // 128             # 8 token tiles
EC = E // 128             # 4 contraction chunks over E
HKD = H * KD              # 512
JC = HKD // 128           # 4 head-pair chunks (2 heads per 128 rows)
HDC = HD // 128           # 16 chunks over the FF hidden dim
NEG = -50.0               # additive mask value (exp(-50) ~ 2e-22)
EPS = 1e-6
HT = NT // 2              # tiles per half


# ---------------------------------------------------------------- device code

class _Pools:
    pass


def _make_pools(ctx, tc):
    p = _Pools()
    p.state = ctx.enter_context(tc.tile_pool(name="state", bufs=1))
    p.aux = ctx.enter_context(tc.tile_pool(name="aux", bufs=1))
    p.memT = ctx.enter_context(tc.tile_pool(name="memT", bufs=1))
    p.xnT = ctx.enter_context(tc.tile_pool(name="xnT", bufs=2))
    p.qT = ctx.enter_context(tc.tile_pool(name="qT", bufs=1))
    p.kT = ctx.enter_context(tc.tile_pool(name="kT", bufs=1))
    p.v = ctx.enter_context(tc.tile_pool(name="v", bufs=1))
    p.oT = ctx.enter_context(tc.tile_pool(name="oT", bufs=1))
    p.otok = ctx.enter_context(tc.tile_pool(name="otok", bufs=1))
    p.hT = ctx.enter_context(tc.tile_pool(name="hT", bufs=1))
    p.xn = ctx.enter_context(tc.tile_pool(name="xn", bufs=3))
    p.pn = ctx.enter_context(tc.tile_pool(name="pn", bufs=3))
    p.stat = ctx.enter_context(tc.tile_pool(name="stat", bufs=8))
    p.mvp = ctx.enter_context(tc.tile_pool(name="mvp", bufs=3))
    p.w = ctx.enter_context(tc.tile_pool(name="w", bufs=12))
    p.const = ctx.enter_context(tc.tile_pool(name="const", bufs=1))
    p.lg = ctx.enter_context(tc.tile_pool(name="lg", bufs=1))
    # PSUM: pproj 3 + ps 2 + psmall 3 = 8 banks
    p.pproj = ctx.enter_context(tc.tile_pool(name="pproj", bufs=3, space="PSUM"))
    p.ps = ctx.enter_context(tc.tile_pool(name="ps", bufs=2, space="PSUM"))
    p.psmall = ctx.enter_context(tc.tile_pool(name="psmall", bufs=3, space="PSUM"))
    return p


def _alloc_mv(p):
    mv = p.mvp.tile([128, NT, 2], F32, tag="mv")
    inv = p.mvp.tile([128, NT], F32, tag="inv")
    return mv, inv


def _emit_ln_stats(nc, p, state, mv, inv, t0, t1):
    """Per-token mean/var for tiles [t0,t1); inv = 1/(sqrt(var*E/(E-1))+eps).

    Matches g*(x-mean)/(sqrt(var_unbiased)+eps+b) with g folded into the
    consumer weights and b == 0.
    """
    for i in range(t0, t1):
        stats = p.stat.tile([128, 6], F32, tag="stats")
        nc.vector.bn_stats(stats[:], state[:, i, :])
        nc.vector.bn_aggr(mv[:, i, :], stats[:])
    std = p.stat.tile([128, t1 - t0], F32, tag="std")
    nc.scalar.activation(std[:], mv[:, t0:t1, 1], AF.Sqrt, scale=float(E) / (E - 1))
    nc.vector.tensor_scalar_add(std[:], std[:], EPS)
    nc.vector.reciprocal(inv[:, t0:t1], std[:])


def _emit_ln_apply(nc, p, state, mv, inv, ident, xnT, t0, t1):
    """xnT[:, c, tile-range] = transposed bf16 (x-mean)*inv for tiles [t0,t1)."""
    for i in range(t0, t1):
        xn = p.xn.tile([128, E], BF16, tag="xn")
        nc.vector.tensor_scalar(
            xn[:], state[:, i, :], mv[:, i, 0:1], inv[:, i:i + 1],
            op0=ALU.subtract, op1=ALU.mult,
        )
        for c in range(EC):
            ptr = p.psmall.tile([128, 128], BF16, tag="psmall")
            nc.tensor.transpose(ptr[:], xn[:, c * 128:(c + 1) * 128], ident[:])
            nc.vector.tensor_copy(xnT[:, c, i * 128:(i + 1) * 128], ptr[:])


def _emit_projT(nc, p, w, xnT, outT, copy_eng, ths):
    """outT[:, j, th*512:...] = (w.T @ xn.T) chunks: [128, JC, T] bf16."""
    for j in range(JC):
        for th in ths:
            ps = p.pproj.tile([128, 512], F32, tag="pproj")
            for c in range(EC):
                nc.tensor.matmul(
                    ps[:],
                    w[:, c, j * 128:(j + 1) * 128],
                    xnT[:, c, th * 512:(th + 1) * 512],
                    start=(c == 0),
                    stop=(c == EC - 1),
                )
            if copy_eng == "act":
                nc.scalar.copy(outT[:, j, th * 512:(th + 1) * 512], ps[:])
            else:
                nc.vector.tensor_copy(outT[:, j, th * 512:(th + 1) * 512], ps[:])


def _emit_proj_tok(nc, p, w, xnT, out_tok, t0, t1, memset=False):
    """Token-major V projection into [128, NT, H, KD+1] bf16 (last col = 1.0,
    so the attention O-matmul also accumulates the softmax denominator)."""
    if memset:
        nc.gpsimd.memset(out_tok[:, :, :, KD:KD + 1], 1.0)
    for i in range(t0, t1):
        ps = p.pproj.tile([128, 512], F32, tag="pproj")
        for c in range(EC):
            nc.tensor.matmul(
                ps[:],
                xnT[:, c, i * 128:(i + 1) * 128],
                w[:, c, :],
                start=(c == 0),
                stop=(c == EC - 1),
            )
        nc.scalar.copy(
            out_tok[:, i, :, 0:KD], ps[:].rearrange("p (h k) -> p h k", k=KD)
        )


def _emit_attn(nc, p, qT, kT, v_tok, o_tok, cmaskT, causal, s0, s1):
    """Attention for sequences [s0,s1) x H heads -> o_tok [128, NT, HKD] bf16.

    Computes S transposed (S^T[m, l] in PSUM), exp on ACT straight to bf16,
    then O[l, kd] = P^T.T @ V with the fused ones-column giving the softmax
    denominator in column KD; normalization is a per-partition scalar mul on
    the O copy-out. causal: adds the [128,128] diagonal-block mask and skips
    fully-masked blocks.
    """
    for s in range(s0, s1):
        for h in range(H):
            j, r = h // 2, (h % 2) * 64
            ptx = p.pn.tile([128, 2, 256], BF16, tag="pn")
            pst = p.ps.tile([128, 2, 256], F32, tag="ps")
            for mc in range(2):
                l0 = 128 * mc if causal else 0
                nc.tensor.matmul(
                    pst[:, mc, l0:256],
                    kT[r:r + 64, j, s * 256 + mc * 128: s * 256 + (mc + 1) * 128],
                    qT[r:r + 64, j, s * 256 + l0:(s + 1) * 256],
                    start=True,
                    stop=True,
                )
                if causal:
                    nc.vector.tensor_tensor(
                        pst[:, mc, l0:l0 + 128], pst[:, mc, l0:l0 + 128],
                        cmaskT[:], op=ALU.add,
                    )
            # one wide exp for both m-chunks (for causal, the never-read
            # [mc=1, l<128] region just exps stale finite psum data)
            nc.scalar.activation(ptx[:, :, :], pst[:, :, :], AF.Exp)
            for lt in range(2):
                mcs = [0] if (causal and lt == 0) else [0, 1]
                pso = p.psmall.tile([128, KD + 1], F32, tag="psmall")
                for n_, mc in enumerate(mcs):
                    nc.tensor.matmul(
                        pso[:],
                        ptx[:, mc, lt * 128:(lt + 1) * 128],
                        v_tok[:, 2 * s + mc, h, :],
                        start=(n_ == 0),
                        stop=(n_ == len(mcs) - 1),
                    )
                inv = p.stat.tile([128, 1], F32, tag="dinv")
                nc.vector.reciprocal(inv[:], pso[:, KD:KD + 1])
                nc.scalar.mul(
                    o_tok[:, 2 * s + lt, h * 64:(h + 1) * 64], pso[:, 0:KD], inv[:]
                )


def _emit_transpose_tok(nc, p, o_tok, oT, ident, t0, t1):
    """o_tok [128, NT, HKD] bf16 -> oT [128, JC, T] bf16 via PE transposes."""
    for i in range(t0, t1):
        for j in range(JC):
            ptr = p.psmall.tile([128, 128], BF16, tag="psmall")
            nc.tensor.transpose(ptr[:], o_tok[:, i, j * 128:(j + 1) * 128], ident[:])
            nc.vector.tensor_copy(oT[:, j, i * 128:(i + 1) * 128], ptr[:])


def _emit_oproj_residual(nc, p, oT, wo, state, base, t0, t1):
    """state[:, i, :] = base[:, i, :] + oT.T @ wo  for tiles [t0,t1)."""
    src = state if base is None else base
    for i in range(t0, t1):
        ps = p.pproj.tile([128, 512], F32, tag="pproj")
        for j in range(JC):
            nc.tensor.matmul(
                ps[:],
                oT[:, j, i * 128:(i + 1) * 128],
                wo[:, j, :],
                start=(j == 0),
                stop=(j == JC - 1),
            )
        nc.vector.tensor_tensor(state[:, i, :], src[:, i, :], ps[:], op=ALU.add)


def _emit_ff1(nc, p, xnT, hT, w1t, c1b, th):
    """hT[:, c2, th-half] = relu(xn @ W1 + c1b) transposed, for one T-half."""
    for c2 in range(HDC):
        ps = p.pproj.tile([128, 512], F32, tag="pproj")
        for c in range(EC):
            nc.tensor.matmul(
                ps[:],
                w1t[c][:, c2 * 128:(c2 + 1) * 128],
                xnT[:, c, th * 512:(th + 1) * 512],
                start=(c == 0),
                stop=(c == EC - 1),
            )
        nc.scalar.activation(
            hT[:, c2, th * 512:(th + 1) * 512], ps[:], AF.Relu,
            bias=c1b[:, c2:c2 + 1],
        )


def _emit_ff2_resid(nc, p, hT, w2t, state, t0, t1):
    """state[:, i, :] += hT.T @ W2  for tiles [t0,t1)."""
    for i in range(t0, t1):
        ps = p.pproj.tile([128, 512], F32, tag="pproj")
        for g in range(4):
            for cc in range(4):
                c2 = g * 4 + cc
                nc.tensor.matmul(
                    ps[:],
                    hT[:, c2, i * 128:(i + 1) * 128],
                    w2t[g][:, cc, :],
                    start=(c2 == 0),
                    stop=(c2 == HDC - 1),
                )
        nc.vector.tensor_tensor(state[:, i, :], state[:, i, :], ps[:], op=ALU.add)


def _load_w4(nc, p, dram, l, idx):
    t = p.w.tile([128, EC, 512], BF16, tag="w")
    nc.sync.dma_start(t[:], dram[l, idx].rearrange("(c p) f -> p c f", p=128))
    return t


def _load_w1(nc, p, dram, l):
    tiles = []
    for c in range(EC):
        t = p.w.tile([128, HD], BF16, tag="w")
        nc.sync.dma_start(t[:], dram[l, c * 128:(c + 1) * 128, :])
        tiles.append(t)
    return tiles


def _load_w2(nc, p, dram, l):
    tiles = []
    for g in range(4):
        t = p.w.tile([128, 4, 512], BF16, tag="w")
        nc.sync.dma_start(
            t[:], dram[l, g * 512:(g + 1) * 512, :].rearrange("(c p) f -> p c f", p=128)
        )
        tiles.append(t)
    return tiles


def _block_attn(nc, p, state, ident, cmask, wq, wk, wv, wo, causal,
                mv, inv, base=None, kvT=None, q_xnT=None):
    """Pre-LN attention block, half-pipelined. mv/inv: stats for LN(state),
    already emitted. kvT: use this [128,EC,T] bf16 source for K/V instead of
    LN(state) (cross-attention / encoder layer 0). q_xnT: use this source for
    Q instead of LN(state) (encoder layer 0).
    Returns (mv2, inv2): stats of the post-residual state for the next block.
    """
    qT = p.qT.tile([128, JC, T], BF16, tag="qT")
    kT = p.kT.tile([128, JC, T], BF16, tag="kT")
    v_tok = p.v.tile([128, NT, H, KD + 1], BF16, tag="v")
    o_tok = p.otok.tile([128, NT, HKD], BF16, tag="otok")
    oT = p.oT.tile([128, JC, T], BF16, tag="oT")

    if q_xnT is not None:
        # encoder layer 0: q from LN(x_mm) (q_xnT), k/v from kvT = LN(x);
        # both already emitted by the caller
        _emit_projT(nc, p, wq, q_xnT, qT, "act", (0, 1))
        _emit_projT(nc, p, wk, kvT, kT, "dve", (0, 1))
        _emit_proj_tok(nc, p, wv, kvT, v_tok, 0, NT, memset=True)
    elif kvT is not None:
        # cross-attention: k/v from kvT (memT), q from LN(state)
        xnT = p.xnT.tile([128, EC, T], BF16, tag="xnT")
        _emit_ln_apply(nc, p, state, mv, inv, ident, xnT, 0, HT)
        _emit_projT(nc, p, wq, xnT, qT, "act", (0,))
        _emit_projT(nc, p, wk, kvT, kT, "dve", (0,))
        _emit_proj_tok(nc, p, wv, kvT, v_tok, 0, HT, memset=True)
        _emit_ln_apply(nc, p, state, mv, inv, ident, xnT, HT, NT)
        _emit_projT(nc, p, wq, xnT, qT, "act", (1,))
        _emit_projT(nc, p, wk, kvT, kT, "dve", (1,))
        _emit_proj_tok(nc, p, wv, kvT, v_tok, HT, NT)
    else:
        # self-attention
        xnT = p.xnT.tile([128, EC, T], BF16, tag="xnT")
        _emit_ln_apply(nc, p, state, mv, inv, ident, xnT, 0, HT)
        _emit_projT(nc, p, wq, xnT, qT, "act", (0,))
        _emit_projT(nc, p, wk, xnT, kT, "dve", (0,))
        _emit_proj_tok(nc, p, wv, xnT, v_tok, 0, HT, memset=True)
        _emit_ln_apply(nc, p, state, mv, inv, ident, xnT, HT, NT)
        _emit_projT(nc, p, wq, xnT, qT, "act", (1,))
        _emit_projT(nc, p, wk, xnT, kT, "dve", (1,))
        _emit_proj_tok(nc, p, wv, xnT, v_tok, HT, NT)

    mv2, inv2 = _alloc_mv(p)
    for s in range(BL):
        _emit_attn(nc, p, qT, kT, v_tok, o_tok, cmask, causal, s, s + 1)
        if s >= 1:
            t0, t1 = 2 * (s - 1), 2 * s
            _emit_transpose_tok(nc, p, o_tok, oT, ident, t0, t1)
            _emit_oproj_residual(nc, p, oT, wo, state, base, t0, t1)
            _emit_ln_stats(nc, p, state, mv2, inv2, t0, t1)
    _emit_transpose_tok(nc, p, o_tok, oT, ident, NT - 2, NT)
    _emit_oproj_residual(nc, p, oT, wo, state, base, NT - 2, NT)
    _emit_ln_stats(nc, p, state, mv2, inv2, NT - 2, NT)
    return mv2, inv2


def _block_ff(nc, p, state, ident, w1t, w2t, c1b, mv, inv):
    """Pre-LN feed-forward block, half-pipelined. Returns next-block stats."""
    xnT = p.xnT.tile([128, EC, T], BF16, tag="xnT")
    hT = p.hT.tile([128, HDC, T], BF16, tag="hT")
    _emit_ln_apply(nc, p, state, mv, inv, ident, xnT, 0, HT)
    _emit_ff1(nc, p, xnT, hT, w1t, c1b, 0)
    _emit_ln_apply(nc, p, state, mv, inv, ident, xnT, HT, NT)
    _emit_ff1(nc, p, xnT, hT, w1t, c1b, 1)
    mv2, inv2 = _alloc_mv(p)
    _emit_ff2_resid(nc, p, hT, w2t, state, 0, HT)
    _emit_ln_stats(nc, p, state, mv2, inv2, 0, HT)
    _emit_ff2_resid(nc, p, hT, w2t, state, HT, NT)
    _emit_ln_stats(nc, p, state, mv2, inv2, HT, NT)
    return mv2, inv2


def build_nc(n_enc=NB, n_dec=NB, out_what="logits"):
    from contextlib import ExitStack

    nc = bacc.Bacc("TRN2", target_bir_lowering=False, debug=False, num_devices=NCORES)
    with ExitStack() as ctx:
        tc = ctx.enter_context(tile.TileContext(nc))
        _emit_model(nc, tc, ctx, n_enc, n_dec, out_what)
    _compile(nc)
    return nc


def _emit_model(nc, tc, ctx, n_enc, n_dec, out_what):
    x0 = nc.dram_tensor("x0", [T, E], F32, kind="ExternalInput").ap()
    xmm0 = nc.dram_tensor("xmm0", [T, E], F32, kind="ExternalInput").ap()
    y0 = nc.dram_tensor("y0", [T, E], F32, kind="ExternalInput").ap()
    encw = nc.dram_tensor("encw", [NB, 4, E, HKD], BF16, kind="ExternalInput").ap()
    encw1 = nc.dram_tensor("encw1", [NB, E, HD], BF16, kind="ExternalInput").ap()
    encw2 = nc.dram_tensor("encw2", [NB, HD, E], BF16, kind="ExternalInput").ap()
    encc1b = nc.dram_tensor("encc1b", [NB, HD], F32, kind="ExternalInput").ap()
    decw = nc.dram_tensor("decw", [NB, 8, E, HKD], BF16, kind="ExternalInput").ap()
    decw1 = nc.dram_tensor("decw1", [NB, E, HD], BF16, kind="ExternalInput").ap()
    decw2 = nc.dram_tensor("decw2", [NB, HD, E], BF16, kind="ExternalInput").ap()
    decc1b = nc.dram_tensor("decc1b", [NB, HD], F32, kind="ExternalInput").ap()
    genw = nc.dram_tensor("genw", [E, VT], BF16, kind="ExternalInput").ap()
    identd = nc.dram_tensor("ident", [128, 128], BF16, kind="ExternalInput").ap()
    cmaskd = nc.dram_tensor("cmask", [128, 128], F32, kind="ExternalInput").ap()

    out_cols = VT if out_what == "logits" else E
    out = nc.dram_tensor("out", [T, out_cols], F32, kind="ExternalOutput").ap()

    p = _make_pools(ctx, tc)

    ident = p.const.tile([128, 128], BF16, tag="ident")
    nc.sync.dma_start(ident[:], identd)
    cmask = p.const.tile([128, 128], F32, tag="cmask")
    nc.sync.dma_start(cmask[:], cmaskd)
    ec1b = p.const.tile([128, NB, HDC], F32, tag="ec1b")
    nc.sync.dma_start(ec1b[:], encc1b.rearrange("l (c q) -> q l c", q=128))
    dc1b = p.const.tile([128, NB, HDC], F32, tag="dc1b")
    nc.sync.dma_start(dc1b[:], decc1b.rearrange("l (c q) -> q l c", q=128))
    genw_sb = p.const.tile([128, EC, VT], BF16, tag="genw")
    nc.sync.dma_start(genw_sb[:], genw.rearrange("(c q) v -> q c v", q=128))

    x_sb = p.state.tile([128, NT, E], F32, tag="x")
    nc.sync.dma_start(x_sb[:], x0.rearrange("(i q) e -> q i e", q=128))
    xmm_sb = p.aux.tile([128, NT, E], F32, tag="aux")
    nc.sync.dma_start(xmm_sb[:], xmm0.rearrange("(i q) e -> q i e", q=128))

    # ------------------------------------------------------------ encoder
    mvx, invx = _alloc_mv(p)
    _emit_ln_stats(nc, p, x_sb, mvx, invx, 0, NT)
    mvm, invm = _alloc_mv(p)
    _emit_ln_stats(nc, p, xmm_sb, mvm, invm, 0, NT)

    mv, inv = mvx, invx
    for l in range(n_enc):
        wq = _load_w4(nc, p, encw, l, 0)
        wk = _load_w4(nc, p, encw, l, 1)
        wv = _load_w4(nc, p, encw, l, 2)
        wo = _load_w4(nc, p, encw, l, 3)
        w1t = _load_w1(nc, p, encw1, l)
        w2t = _load_w2(nc, p, encw2, l)

        if l == 0:
            q_xnT = p.xnT.tile([128, EC, T], BF16, tag="xnT")
            _emit_ln_apply(nc, p, xmm_sb, mvm, invm, ident, q_xnT, 0, NT)
            kvT = p.xnT.tile([128, EC, T], BF16, tag="xnT")
            _emit_ln_apply(nc, p, x_sb, mvx, invx, ident, kvT, 0, NT)
            mv, inv = _block_attn(nc, p, x_sb, ident, cmask, wq, wk, wv, wo,
                                  False, None, None, base=xmm_sb,
                                  kvT=kvT, q_xnT=q_xnT)
        else:
            mv, inv = _block_attn(nc, p, x_sb, ident, cmask, wq, wk, wv, wo,
                                  False, mv, inv)
        mv, inv = _block_ff(nc, p, x_sb, ident, w1t, w2t, ec1b[:, l, :], mv, inv)

    if out_what == "enc_state":
        nc.sync.dma_start(out.rearrange("(i q) e -> q i e", q=128), x_sb[:])
        return

    # encoder final norm -> memT (gamma folded into dec cross K/V weights)
    memT = p.memT.tile([128, EC, T], BF16, tag="memT")
    _emit_ln_apply(nc, p, x_sb, mv, inv, ident, memT, 0, NT)

    y_sb = p.aux.tile([128, NT, E], F32, tag="aux")
    nc.sync.dma_start(y_sb[:], y0.rearrange("(i q) e -> q i e", q=128))

    # ------------------------------------------------------------ decoder
    mv, inv = _alloc_mv(p)
    _emit_ln_stats(nc, p, y_sb, mv, inv, 0, NT)
    for l in range(n_dec):
        wq1 = _load_w4(nc, p, decw, l, 0)
        wk1 = _load_w4(nc, p, decw, l, 1)
        wv1 = _load_w4(nc, p, decw, l, 2)
        wo1 = _load_w4(nc, p, decw, l, 3)
        wq2 = _load_w4(nc, p, decw, l, 4)
        wk2 = _load_w4(nc, p, decw, l, 5)
        wv2 = _load_w4(nc, p, decw, l, 6)
        wo2 = _load_w4(nc, p, decw, l, 7)
        w1t = _load_w1(nc, p, decw1, l)
        w2t = _load_w2(nc, p, decw2, l)

        mv, inv = _block_attn(nc, p, y_sb, ident, cmask, wq1, wk1, wv1, wo1,
                              True, mv, inv)
        mv, inv = _block_attn(nc, p, y_sb, ident, cmask, wq2, wk2, wv2, wo2,
                              False, mv, inv, kvT=memT)
        mv, inv = _block_ff(nc, p, y_sb, ident, w1t, w2t, dc1b[:, l, :], mv, inv)

    if out_what == "dec_state":
        nc.sync.dma_start(out.rearrange("(i q) e -> q i e", q=128), y_sb[:])
        return

    # final norm + generator (dec_norm gamma folded into genw; gen_b == 0)
    ynT = p.xnT.tile([128, EC, T], BF16, tag="xnT")
    _emit_ln_apply(nc, p, y_sb, mv, inv, ident, ynT, 0, NT)
    lg = p.lg.tile([128, NT, VT], F32, tag="lg")
    for i in range(NT):
        ps = p.pproj.tile([128, 512], F32, tag="pproj")
        for c in range(EC):
            nc.tensor.matmul(
                ps[:, :VT],
                ynT[:, c, i * 128:(i + 1) * 128],
                genw_sb[:, c, :],
                start=(c == 0),
                stop=(c == EC - 1),
            )
        nc.vector.tensor_copy(lg[:, i, :], ps[:, :VT])
    nc.sync.dma_start(out.rearrange("(i q) v -> q i v", q=128), lg[:])


def _compile(nc):
    import bass_rust

    nc.compile()
    # Bacc.compile() can leave instructions with more embedded sync waits
    # than walrus accepts; one more split pass cleans them up.
    bass_rust.generate_event_semaphores(nc)


# ------------------------------------------------------------------ host prep

def _pos_table(n):
    pos = np.arange(n, dtype=np.float32)[:, None]
    bins = np.arange(0, E, 2, dtype=np.float32)[None, :]
    ang = (pos / np.power(10000.0, bins / E)).astype(np.float32)
    return np.stack([np.sin(ang), np.cos(ang)], axis=2).reshape(n, E).astype(np.float32)


def _cat_heads(w):
    # [H, E, KD] -> [E, H*KD]
    return np.transpose(np.asarray(w, np.float32), (1, 0, 2)).reshape(E, HKD)


def _host_inputs(src, tgt, fp, src_mask, tgt_mask, params):
    f32 = np.float32
    src = np.asarray(src)
    tgt = np.asarray(tgt)
    fp = np.asarray(fp)
    pe = _pos_table(L)
    scale = math.sqrt(E)

    x0 = np.asarray(params["src_emb"], f32)[src] * scale + pe[None, :, :]
    xmm0 = np.asarray(params["fp_emb"], f32)[fp] * scale + pe[None, :, :]
    y0 = np.asarray(params["tgt_emb"], f32)[tgt] * scale + pe[None, :, :]

    sc = 1.0 / math.sqrt(KD)
    encw = np.zeros((NB, 4, E, HKD), f32)
    encw1 = np.zeros((NB, E, HD), f32)
    encw2 = np.zeros((NB, HD, E), f32)
    encc1b = np.zeros((NB, HD), f32)
    for l, pl in enumerate(params["enc"]):
        g1 = np.asarray(pl["g1"], f32)
        g2 = np.asarray(pl["g2"], f32)
        encw[l, 0] = _cat_heads(pl["Wq"]) * g1[:, None] * sc
        encw[l, 1] = _cat_heads(pl["Wk"]) * g1[:, None]
        encw[l, 2] = _cat_heads(pl["Wv"]) * g1[:, None]
        encw[l, 3] = np.asarray(pl["Wo"], f32)
        encw1[l] = np.asarray(pl["W1"], f32) * g2[:, None]
        encw2[l] = np.asarray(pl["W2"], f32)
        encc1b[l] = np.asarray(pl["c1b"], f32)

    eng = np.asarray(params["enc_norm_g"], f32)
    decw = np.zeros((NB, 8, E, HKD), f32)
    decw1 = np.zeros((NB, E, HD), f32)
    decw2 = np.zeros((NB, HD, E), f32)
    decc1b = np.zeros((NB, HD), f32)
    for l, pl in enumerate(params["dec"]):
        g1 = np.asarray(pl["g1"], f32)
        g2 = np.asarray(pl["g2"], f32)
        g3 = np.asarray(pl["g3"], f32)
        decw[l, 0] = _cat_heads(pl["Wq1"]) * g1[:, None] * sc
        decw[l, 1] = _cat_heads(pl["Wk1"]) * g1[:, None]
        decw[l, 2] = _cat_heads(pl["Wv1"]) * g1[:, None]
        decw[l, 3] = np.asarray(pl["Wo1"], f32)
        decw[l, 4] = _cat_heads(pl["Wq2"]) * g2[:, None] * sc
        decw[l, 5] = _cat_heads(pl["Wk2"]) * eng[:, None]
        decw[l, 6] = _cat_heads(pl["Wv2"]) * eng[:, None]
        decw[l, 7] = np.asarray(pl["Wo2"], f32)
        decw1[l] = np.asarray(pl["W1"], f32) * g3[:, None]
        decw2[l] = np.asarray(pl["W2"], f32)
        decc1b[l] = np.asarray(pl["c1b"], f32)

    dng = np.asarray(params["dec_norm_g"], f32)
    genw = np.asarray(params["gen_W"], f32) * dng[:, None]

    ident = np.eye(128, dtype=NPBF16)
    # transposed diagonal-block causal mask: mask[m, l] = 0 if m <= l else NEG
    mm_ = np.arange(128)[:, None]
    ll_ = np.arange(128)[None, :]
    cmask = np.where(mm_ <= ll_, 0.0, NEG).astype(f32)

    shared = dict(
        encw=encw.astype(NPBF16), encw1=encw1.astype(NPBF16),
        encw2=encw2.astype(NPBF16), encc1b=encc1b,
        decw=decw.astype(NPBF16), decw1=decw1.astype(NPBF16),
        decw2=decw2.astype(NPBF16), decc1b=decc1b,
        genw=genw.astype(NPBF16), ident=ident, cmask=cmask,
    )
    in_maps = []
    for i in range(NCORES):
        sl = slice(i * BL, (i + 1) * BL)
        m = dict(shared)
        m["x0"] = np.ascontiguousarray(x0[sl].reshape(T, E))
        m["xmm0"] = np.ascontiguousarray(xmm0[sl].reshape(T, E))
        m["y0"] = np.ascontiguousarray(y0[sl].reshape(T, E))
        in_maps.append(m)
    return in_maps


_NC_CACHE = {}


def _get_nc(n_enc=NB, n_dec=NB, out_what="logits"):
    key = (n_enc, n_dec, out_what)
    if key not in _NC_CACHE:
        _NC_CACHE[key] = build_nc(n_enc, n_dec, out_what)
    return _NC_CACHE[key]


def kernel(src, tgt, fp, src_mask, tgt_mask, params):
    nc = _get_nc()
    in_maps = _host_inputs(src, tgt, fp, src_mask, tgt_mask, params)
    res = bass_utils.run_bass_kernel_spmd(nc, in_maps, list(range(NCORES)))
    outs = [res.results[i]["out"].reshape(BL, L, VT) for i in range(NCORES)]
    return np.concatenate(outs, axis=0)


# revision 12
# speedup vs baseline: 1.1085x; 1.1085x over previous
"""Trainium2 Bass kernel for nn_AugmentedTransformer (encoder-decoder transformer).

Sharding: data-parallel over batch. B=32 sequences -> 4 per core x 8 cores.
Each core runs the full 6+6-layer model on its 4 sequences (1024 tokens).
Weights are bf16-cast host-side and replicated to every core; all matmuls run
in bf16 with fp32 PSUM accumulation; layernorm / softmax statistics in fp32.

Host-side prep (inside kernel()):
  - embedding gathers + positional encoding -> dense f32 activations
  - LN gammas, the 1/sqrt(64) attention scale, and the encoder-final-norm
    gamma are folded into the adjacent weight matrices (exact folds).
  - LN beta / conv bias c2b / gen_b are zeros in this problem's setup and the
    LN denominators' beta is zero too; c1b is applied exactly via the Relu
    activation per-partition bias. src/tgt masks are all-ones by spec
    (fill: ones); the decoder causal mask is applied as a static additive
    mask before the exp.

Device schedule: activations stay resident in SBUF. Attention computes S
transposed (softmax denominator fused into the O matmul via a ones-column in
V), so no probability transposes are needed. All phases are half-pipelined
over token tiles so the PE never stalls on LN statistics (which are computed
during the previous block's matmuls) -- this keeps the PE HAM clock at 2.4GHz.
"""

import math
import numpy as np
import ml_dtypes

import concourse.bass as bass
import concourse.tile as tile
import concourse.mybir as mybir
from concourse import bacc
import concourse.bass_utils as bass_utils

dt = mybir.dt
F32 = dt.float32
BF16 = dt.bfloat16
AF = mybir.ActivationFunctionType
ALU = mybir.AluOpType
NPBF16 = ml_dtypes.bfloat16

B, L, E, KD, H, HD, NB, VT = 32, 256, 512, 64, 8, 2048, 6, 66
NCORES = 8
BL = B // NCORES          # 4 sequences per core
T = BL * L                # 1024 tokens per core
NT = T // 128             # 8 token tiles
EC = E // 128             # 4 contraction chunks over E
HKD = H * KD              # 512
JC = HKD // 128           # 4 head-pair chunks (2 heads per 128 rows)
HDC = HD // 128           # 16 chunks over the FF hidden dim
NEG = -50.0               # additive mask value (exp(-50) ~ 2e-22)
EPS = 1e-6
HT = NT // 2              # tiles per half


# ---------------------------------------------------------------- device code

class _Pools:
    pass


def _make_pools(ctx, tc):
    p = _Pools()
    p.state = ctx.enter_context(tc.tile_pool(name="state", bufs=1))
    p.aux = ctx.enter_context(tc.tile_pool(name="aux", bufs=1))
    p.memT = ctx.enter_context(tc.tile_pool(name="memT", bufs=1))
    p.xnT = ctx.enter_context(tc.tile_pool(name="xnT", bufs=2))
    p.qT = ctx.enter_context(tc.tile_pool(name="qT", bufs=1))
    p.kT = ctx.enter_context(tc.tile_pool(name="kT", bufs=1))
    p.v = ctx.enter_context(tc.tile_pool(name="v", bufs=1))
    p.oT = ctx.enter_context(tc.tile_pool(name="oT", bufs=1))
    p.otok = ctx.enter_context(tc.tile_pool(name="otok", bufs=1))
    p.hT = ctx.enter_context(tc.tile_pool(name="hT", bufs=1))
    p.xn = ctx.enter_context(tc.tile_pool(name="xn", bufs=3))
    p.pn = ctx.enter_context(tc.tile_pool(name="pn", bufs=3))
    p.stat = ctx.enter_context(tc.tile_pool(name="stat", bufs=8))
    p.mvp = ctx.enter_context(tc.tile_pool(name="mvp", bufs=3))
    p.w = ctx.enter_context(tc.tile_pool(name="w", bufs=12))
    p.const = ctx.enter_context(tc.tile_pool(name="const", bufs=1))
    p.lg = ctx.enter_context(tc.tile_pool(name="lg", bufs=1))
    # PSUM: pproj 3 + ps 2 + psmall 3 = 8 banks
    p.pproj = ctx.enter_context(tc.tile_pool(name="pproj", bufs=3, space="PSUM"))
    p.ps = ctx.enter_context(tc.tile_pool(name="ps", bufs=2, space="PSUM"))
    p.psmall = ctx.enter_context(tc.tile_pool(name="psmall", bufs=3, space="PSUM"))
    return p


def _alloc_mv(p):
    mv = p.mvp.tile([128, NT, 2], F32, tag="mv")
    inv = p.mvp.tile([128, NT], F32, tag="inv")
    return mv, inv


def _emit_ln_stats(nc, p, state, mv, inv, t0, t1):
    """Per-token mean/var for tiles [t0,t1); inv = 1/(sqrt(var*E/(E-1))+eps).

    Matches g*(x-mean)/(sqrt(var_unbiased)+eps+b) with g folded into the
    consumer weights and b == 0.
    """
    for i in range(t0, t1):
        stats = p.stat.tile([128, 6], F32, tag="stats")
        nc.vector.bn_stats(stats[:], state[:, i, :])
        nc.vector.bn_aggr(mv[:, i, :], stats[:])
    std = p.stat.tile([128, t1 - t0], F32, tag="std")
    nc.scalar.activation(std[:], mv[:, t0:t1, 1], AF.Sqrt, scale=float(E) / (E - 1))
    nc.vector.tensor_scalar_add(std[:], std[:], EPS)
    nc.vector.reciprocal(inv[:, t0:t1], std[:])


def _emit_ln_apply(nc, p, state, mv, inv, ident, xnT, t0, t1):
    """xnT[:, c, tile-range] = transposed bf16 (x-mean)*inv for tiles [t0,t1)."""
    for i in range(t0, t1):
        xn = p.xn.tile([128, E], BF16, tag="xn")
        nc.vector.tensor_scalar(
            xn[:], state[:, i, :], mv[:, i, 0:1], inv[:, i:i + 1],
            op0=ALU.subtract, op1=ALU.mult,
        )
        for c in range(EC):
            ptr = p.psmall.tile([128, 128], BF16, tag="psmall")
            nc.tensor.transpose(ptr[:], xn[:, c * 128:(c + 1) * 128], ident[:])
            nc.vector.tensor_copy(xnT[:, c, i * 128:(i + 1) * 128], ptr[:])


def _emit_projT(nc, p, w, xnT, outT, copy_eng, ths):
    """outT[:, j, th*512:...] = (w.T @ xn.T) chunks: [128, JC, T] bf16."""
    for j in range(JC):
        for th in ths:
            ps = p.pproj.tile([128, 512], F32, tag="pproj")
            for c in range(EC):
                nc.tensor.matmul(
                    ps[:],
                    w[:, c, j * 128:(j + 1) * 128],
                    xnT[:, c, th * 512:(th + 1) * 512],
                    start=(c == 0),
                    stop=(c == EC - 1),
                )
            if copy_eng == "act":
                nc.scalar.copy(outT[:, j, th * 512:(th + 1) * 512], ps[:])
            else:
                nc.vector.tensor_copy(outT[:, j, th * 512:(th + 1) * 512], ps[:])


def _emit_proj_tok(nc, p, w, xnT, out_tok, t0, t1, memset=False):
    """Token-major V projection into [128, NT, H, KD+1] bf16 (last col = 1.0,
    so the attention O-matmul also accumulates the softmax denominator)."""
    if memset:
        nc.gpsimd.memset(out_tok[:, :, :, KD:KD + 1], 1.0)
    for i in range(t0, t1):
        ps = p.pproj.tile([128, 512], F32, tag="pproj")
        for c in range(EC):
            nc.tensor.matmul(
                ps[:],
                xnT[:, c, i * 128:(i + 1) * 128],
                w[:, c, :],
                start=(c == 0),
                stop=(c == EC - 1),
            )
        nc.scalar.copy(
            out_tok[:, i, :, 0:KD], ps[:].rearrange("p (h k) -> p h k", k=KD)
        )


def _emit_attn(nc, p, qT, kT, v_tok, o_tok, cmaskT, causal, s0, s1):
    """Attention for sequences [s0,s1) x H heads -> o_tok [128, NT, HKD] bf16.

    Computes S transposed (S^T[m, l] in PSUM), exp on ACT straight to bf16,
    then O[l, kd] = P^T.T @ V with the fused ones-column giving the softmax
    denominator in column KD; normalization is a per-partition scalar mul on
    the O copy-out. causal: adds the [128,128] diagonal-block mask and skips
    fully-masked blocks.
    """
    for s in range(s0, s1):
        for h in range(H):
            j, r = h // 2, (h % 2) * 64
            ptx = p.pn.tile([128, 2, 256], BF16, tag="pn")
            pst = p.ps.tile([128, 2, 256], F32, tag="ps")
            for mc in range(2):
                l0 = 128 * mc if causal else 0
                nc.tensor.matmul(
                    pst[:, mc, l0:256],
                    kT[r:r + 64, j, s * 256 + mc * 128: s * 256 + (mc + 1) * 128],
                    qT[r:r + 64, j, s * 256 + l0:(s + 1) * 256],
                    start=True,
                    stop=True,
                )
                if causal:
                    nc.vector.tensor_tensor(
                        pst[:, mc, l0:l0 + 128], pst[:, mc, l0:l0 + 128],
                        cmaskT[:], op=ALU.add,
                    )
            # one wide exp for both m-chunks (for causal, the never-read
            # [mc=1, l<128] region just exps stale finite psum data)
            nc.scalar.activation(ptx[:, :, :], pst[:, :, :], AF.Exp)
            for lt in range(2):
                mcs = [0] if (causal and lt == 0) else [0, 1]
                pso = p.psmall.tile([128, KD + 1], F32, tag="psmall")
                for n_, mc in enumerate(mcs):
                    nc.tensor.matmul(
                        pso[:],
                        ptx[:, mc, lt * 128:(lt + 1) * 128],
                        v_tok[:, 2 * s + mc, h, :],
                        start=(n_ == 0),
                        stop=(n_ == len(mcs) - 1),
                    )
                inv = p.stat.tile([128, 1], F32, tag="dinv")
                nc.vector.reciprocal(inv[:], pso[:, KD:KD + 1])
                nc.vector.tensor_scalar_mul(
                    o_tok[:, 2 * s + lt, h * 64:(h + 1) * 64], pso[:, 0:KD], inv[:]
                )


def _emit_transpose_tok(nc, p, o_tok, oT, ident, t0, t1):
    """o_tok [128, NT, HKD] bf16 -> oT [128, JC, T] bf16 via PE transposes."""
    for i in range(t0, t1):
        for j in range(JC):
            ptr = p.psmall.tile([128, 128], BF16, tag="psmall")
            nc.tensor.transpose(ptr[:], o_tok[:, i, j * 128:(j + 1) * 128], ident[:])
            nc.vector.tensor_copy(oT[:, j, i * 128:(i + 1) * 128], ptr[:])


def _emit_oproj_residual(nc, p, oT, wo, state, base, t0, t1):
    """state[:, i, :] = base[:, i, :] + oT.T @ wo  for tiles [t0,t1)."""
    src = state if base is None else base
    for i in range(t0, t1):
        ps = p.pproj.tile([128, 512], F32, tag="pproj")
        for j in range(JC):
            nc.tensor.matmul(
                ps[:],
                oT[:, j, i * 128:(i + 1) * 128],
                wo[:, j, :],
                start=(j == 0),
                stop=(j == JC - 1),
            )
        nc.vector.tensor_tensor(state[:, i, :], src[:, i, :], ps[:], op=ALU.add)


def _emit_ff1(nc, p, xnT, hT, w1t, c1b, th):
    """hT[:, c2, th-half] = relu(xn @ W1 + c1b) transposed, for one T-half."""
    for c2 in range(HDC):
        ps = p.pproj.tile([128, 512], F32, tag="pproj")
        for c in range(EC):
            nc.tensor.matmul(
                ps[:],
                w1t[c][:, c2 * 128:(c2 + 1) * 128],
                xnT[:, c, th * 512:(th + 1) * 512],
                start=(c == 0),
                stop=(c == EC - 1),
            )
        nc.scalar.activation(
            hT[:, c2, th * 512:(th + 1) * 512], ps[:], AF.Relu,
            bias=c1b[:, c2:c2 + 1],
        )


def _emit_ff2_resid(nc, p, hT, w2t, state, t0, t1):
    """state[:, i, :] += hT.T @ W2  for tiles [t0,t1)."""
    for i in range(t0, t1):
        ps = p.pproj.tile([128, 512], F32, tag="pproj")
        for g in range(4):
            for cc in range(4):
                c2 = g * 4 + cc
                nc.tensor.matmul(
                    ps[:],
                    hT[:, c2, i * 128:(i + 1) * 128],
                    w2t[g][:, cc, :],
                    start=(c2 == 0),
                    stop=(c2 == HDC - 1),
                )
        nc.vector.tensor_tensor(state[:, i, :], state[:, i, :], ps[:], op=ALU.add)


def _load_w4(nc, p, dram, l, idx):
    t = p.w.tile([128, EC, 512], BF16, tag="w")
    nc.sync.dma_start(t[:], dram[l, idx].rearrange("(c p) f -> p c f", p=128))
    return t


def _load_w1(nc, p, dram, l):
    tiles = []
    for c in range(EC):
        t = p.w.tile([128, HD], BF16, tag="w")
        nc.sync.dma_start(t[:], dram[l, c * 128:(c + 1) * 128, :])
        tiles.append(t)
    return tiles


def _load_w2(nc, p, dram, l):
    tiles = []
    for g in range(4):
        t = p.w.tile([128, 4, 512], BF16, tag="w")
        nc.sync.dma_start(
            t[:], dram[l, g * 512:(g + 1) * 512, :].rearrange("(c p) f -> p c f", p=128)
        )
        tiles.append(t)
    return tiles


def _block_attn(nc, p, state, ident, cmask, wq, wk, wv, wo, causal,
                mv, inv, base=None, kvT=None, q_xnT=None):
    """Pre-LN attention block, half-pipelined. mv/inv: stats for LN(state),
    already emitted. kvT: use this [128,EC,T] bf16 source for K/V instead of
    LN(state) (cross-attention / encoder layer 0). q_xnT: use this source for
    Q instead of LN(state) (encoder layer 0).
    Returns (mv2, inv2): stats of the post-residual state for the next block.
    """
    qT = p.qT.tile([128, JC, T], BF16, tag="qT")
    kT = p.kT.tile([128, JC, T], BF16, tag="kT")
    v_tok = p.v.tile([128, NT, H, KD + 1], BF16, tag="v")
    o_tok = p.otok.tile([128, NT, HKD], BF16, tag="otok")
    oT = p.oT.tile([128, JC, T], BF16, tag="oT")

    if q_xnT is not None:
        # encoder layer 0: q from LN(x_mm) (q_xnT), k/v from kvT = LN(x);
        # both already emitted by the caller
        _emit_projT(nc, p, wq, q_xnT, qT, "act", (0, 1))
        _emit_projT(nc, p, wk, kvT, kT, "dve", (0, 1))
        _emit_proj_tok(nc, p, wv, kvT, v_tok, 0, NT, memset=True)
    elif kvT is not None:
        # cross-attention: k/v from kvT (memT), q from LN(state)
        xnT = p.xnT.tile([128, EC, T], BF16, tag="xnT")
        _emit_ln_apply(nc, p, state, mv, inv, ident, xnT, 0, HT)
        _emit_projT(nc, p, wq, xnT, qT, "act", (0,))
        _emit_projT(nc, p, wk, kvT, kT, "dve", (0,))
        _emit_proj_tok(nc, p, wv, kvT, v_tok, 0, HT, memset=True)
        _emit_ln_apply(nc, p, state, mv, inv, ident, xnT, HT, NT)
        _emit_projT(nc, p, wq, xnT, qT, "act", (1,))
        _emit_projT(nc, p, wk, kvT, kT, "dve", (1,))
        _emit_proj_tok(nc, p, wv, kvT, v_tok, HT, NT)
    else:
        # self-attention
        xnT = p.xnT.tile([128, EC, T], BF16, tag="xnT")
        _emit_ln_apply(nc, p, state, mv, inv, ident, xnT, 0, HT)
        _emit_projT(nc, p, wq, xnT, qT, "act", (0,))
        _emit_projT(nc, p, wk, xnT, kT, "dve", (0,))
        _emit_proj_tok(nc, p, wv, xnT, v_tok, 0, HT, memset=True)
        _emit_ln_apply(nc, p, state, mv, inv, ident, xnT, HT, NT)
        _emit_projT(nc, p, wq, xnT, qT, "act", (1,))
        _emit_projT(nc, p, wk, xnT, kT, "dve", (1,))
        _emit_proj_tok(nc, p, wv, xnT, v_tok, HT, NT)

    mv2, inv2 = _alloc_mv(p)
    for s in range(BL):
        _emit_attn(nc, p, qT, kT, v_tok, o_tok, cmask, causal, s, s + 1)
        if s >= 1:
            t0, t1 = 2 * (s - 1), 2 * s
            _emit_transpose_tok(nc, p, o_tok, oT, ident, t0, t1)
            _emit_oproj_residual(nc, p, oT, wo, state, base, t0, t1)
            _emit_ln_stats(nc, p, state, mv2, inv2, t0, t1)
    _emit_transpose_tok(nc, p, o_tok, oT, ident, NT - 2, NT)
    _emit_oproj_residual(nc, p, oT, wo, state, base, NT - 2, NT)
    _emit_ln_stats(nc, p, state, mv2, inv2, NT - 2, NT)
    return mv2, inv2


def _block_ff(nc, p, state, ident, w1t, w2t, c1b, mv, inv):
    """Pre-LN feed-forward block, half-pipelined. Returns next-block stats."""
    xnT = p.xnT.tile([128, EC, T], BF16, tag="xnT")
    hT = p.hT.tile([128, HDC, T], BF16, tag="hT")
    _emit_ln_apply(nc, p, state, mv, inv, ident, xnT, 0, HT)
    _emit_ff1(nc, p, xnT, hT, w1t, c1b, 0)
    _emit_ln_apply(nc, p, state, mv, inv, ident, xnT, HT, NT)
    _emit_ff1(nc, p, xnT, hT, w1t, c1b, 1)
    mv2, inv2 = _alloc_mv(p)
    _emit_ff2_resid(nc, p, hT, w2t, state, 0, HT)
    _emit_ln_stats(nc, p, state, mv2, inv2, 0, HT)
    _emit_ff2_resid(nc, p, hT, w2t, state, HT, NT)
    _emit_ln_stats(nc, p, state, mv2, inv2, HT, NT)
    return mv2, inv2


def build_nc(n_enc=NB, n_dec=NB, out_what="logits"):
    from contextlib import ExitStack

    nc = bacc.Bacc("TRN2", target_bir_lowering=False, debug=False, num_devices=NCORES)
    with ExitStack() as ctx:
        tc = ctx.enter_context(tile.TileContext(nc))
        _emit_model(nc, tc, ctx, n_enc, n_dec, out_what)
    _compile(nc)
    return nc


def _emit_model(nc, tc, ctx, n_enc, n_dec, out_what):
    x0 = nc.dram_tensor("x0", [T, E], F32, kind="ExternalInput").ap()
    xmm0 = nc.dram_tensor("xmm0", [T, E], F32, kind="ExternalInput").ap()
    y0 = nc.dram_tensor("y0", [T, E], F32, kind="ExternalInput").ap()
    encw = nc.dram_tensor("encw", [NB, 4, E, HKD], BF16, kind="ExternalInput").ap()
    encw1 = nc.dram_tensor("encw1", [NB, E, HD], BF16, kind="ExternalInput").ap()
    encw2 = nc.dram_tensor("encw2", [NB, HD, E], BF16, kind="ExternalInput").ap()
    encc1b = nc.dram_tensor("encc1b", [NB, HD], F32, kind="ExternalInput").ap()
    decw = nc.dram_tensor("decw", [NB, 8, E, HKD], BF16, kind="ExternalInput").ap()
    decw1 = nc.dram_tensor("decw1", [NB, E, HD], BF16, kind="ExternalInput").ap()
    decw2 = nc.dram_tensor("decw2", [NB, HD, E], BF16, kind="ExternalInput").ap()
    decc1b = nc.dram_tensor("decc1b", [NB, HD], F32, kind="ExternalInput").ap()
    genw = nc.dram_tensor("genw", [E, VT], BF16, kind="ExternalInput").ap()
    identd = nc.dram_tensor("ident", [128, 128], BF16, kind="ExternalInput").ap()
    cmaskd = nc.dram_tensor("cmask", [128, 128], F32, kind="ExternalInput").ap()

    out_cols = VT if out_what == "logits" else E
    out = nc.dram_tensor("out", [T, out_cols], F32, kind="ExternalOutput").ap()

    p = _make_pools(ctx, tc)

    ident = p.const.tile([128, 128], BF16, tag="ident")
    nc.sync.dma_start(ident[:], identd)
    cmask = p.const.tile([128, 128], F32, tag="cmask")
    nc.sync.dma_start(cmask[:], cmaskd)
    ec1b = p.const.tile([128, NB, HDC], F32, tag="ec1b")
    nc.sync.dma_start(ec1b[:], encc1b.rearrange("l (c q) -> q l c", q=128))
    dc1b = p.const.tile([128, NB, HDC], F32, tag="dc1b")
    nc.sync.dma_start(dc1b[:], decc1b.rearrange("l (c q) -> q l c", q=128))
    genw_sb = p.const.tile([128, EC, VT], BF16, tag="genw")
    nc.sync.dma_start(genw_sb[:], genw.rearrange("(c q) v -> q c v", q=128))

    x_sb = p.state.tile([128, NT, E], F32, tag="x")
    nc.sync.dma_start(x_sb[:], x0.rearrange("(i q) e -> q i e", q=128))
    xmm_sb = p.aux.tile([128, NT, E], F32, tag="aux")
    nc.sync.dma_start(xmm_sb[:], xmm0.rearrange("(i q) e -> q i e", q=128))

    # ------------------------------------------------------------ encoder
    mvx, invx = _alloc_mv(p)
    _emit_ln_stats(nc, p, x_sb, mvx, invx, 0, NT)
    mvm, invm = _alloc_mv(p)
    _emit_ln_stats(nc, p, xmm_sb, mvm, invm, 0, NT)

    mv, inv = mvx, invx
    for l in range(n_enc):
        wq = _load_w4(nc, p, encw, l, 0)
        wk = _load_w4(nc, p, encw, l, 1)
        wv = _load_w4(nc, p, encw, l, 2)
        wo = _load_w4(nc, p, encw, l, 3)
        w1t = _load_w1(nc, p, encw1, l)
        w2t = _load_w2(nc, p, encw2, l)

        if l == 0:
            q_xnT = p.xnT.tile([128, EC, T], BF16, tag="xnT")
            _emit_ln_apply(nc, p, xmm_sb, mvm, invm, ident, q_xnT, 0, NT)
            kvT = p.xnT.tile([128, EC, T], BF16, tag="xnT")
            _emit_ln_apply(nc, p, x_sb, mvx, invx, ident, kvT, 0, NT)
            mv, inv = _block_attn(nc, p, x_sb, ident, cmask, wq, wk, wv, wo,
                                  False, None, None, base=xmm_sb,
                                  kvT=kvT, q_xnT=q_xnT)
        else:
            mv, inv = _block_attn(nc, p, x_sb, ident, cmask, wq, wk, wv, wo,
                                  False, mv, inv)
        mv, inv = _block_ff(nc, p, x_sb, ident, w1t, w2t, ec1b[:, l, :], mv, inv)

    if out_what == "enc_state":
        nc.sync.dma_start(out.rearrange("(i q) e -> q i e", q=128), x_sb[:])
        return

    # encoder final norm -> memT (gamma folded into dec cross K/V weights)
    memT = p.memT.tile([128, EC, T], BF16, tag="memT")
    _emit_ln_apply(nc, p, x_sb, mv, inv, ident, memT, 0, NT)

    y_sb = p.aux.tile([128, NT, E], F32, tag="aux")
    nc.sync.dma_start(y_sb[:], y0.rearrange("(i q) e -> q i e", q=128))

    # ------------------------------------------------------------ decoder
    mv, inv = _alloc_mv(p)
    _emit_ln_stats(nc, p, y_sb, mv, inv, 0, NT)
    for l in range(n_dec):
        wq1 = _load_w4(nc, p, decw, l, 0)
        wk1 = _load_w4(nc, p, decw, l, 1)
        wv1 = _load_w4(nc, p, decw, l, 2)
        wo1 = _load_w4(nc, p, decw, l, 3)
        wq2 = _load_w4(nc, p, decw, l, 4)
        wk2 = _load_w4(nc, p, decw, l, 5)
        wv2 = _load_w4(nc, p, decw, l, 6)
        wo2 = _load_w4(nc, p, decw, l, 7)
        w1t = _load_w1(nc, p, decw1, l)
        w2t = _load_w2(nc, p, decw2, l)

        mv, inv = _block_attn(nc, p, y_sb, ident, cmask, wq1, wk1, wv1, wo1,
                              True, mv, inv)
        mv, inv = _block_attn(nc, p, y_sb, ident, cmask, wq2, wk2, wv2, wo2,
                              False, mv, inv, kvT=memT)
        mv, inv = _block_ff(nc, p, y_sb, ident, w1t, w2t, dc1b[:, l, :], mv, inv)

    if out_what == "dec_state":
        nc.sync.dma_start(out.rearrange("(i q) e -> q i e", q=128), y_sb[:])
        return

    # final norm + generator (dec_norm gamma folded into genw; gen_b == 0)
    ynT = p.xnT.tile([128, EC, T], BF16, tag="xnT")
    _emit_ln_apply(nc, p, y_sb, mv, inv, ident, ynT, 0, NT)
    lg = p.lg.tile([128, NT, VT], F32, tag="lg")
    for i in range(NT):
        ps = p.pproj.tile([128, 512], F32, tag="pproj")
        for c in range(EC):
            nc.tensor.matmul(
                ps[:, :VT],
                ynT[:, c, i * 128:(i + 1) * 128],
                genw_sb[:, c, :],
                start=(c == 0),
                stop=(c == EC - 1),
            )
        nc.vector.tensor_copy(lg[:, i, :], ps[:, :VT])
    nc.sync.dma_start(out.rearrange("(i q) v -> q i v", q=128), lg[:])


def _compile(nc):
    import bass_rust

    nc.compile()
    # Bacc.compile() can leave instructions with more embedded sync waits
    # than walrus accepts; one more split pass cleans them up.
    bass_rust.generate_event_semaphores(nc)


# ------------------------------------------------------------------ host prep

def _pos_table(n):
    pos = np.arange(n, dtype=np.float32)[:, None]
    bins = np.arange(0, E, 2, dtype=np.float32)[None, :]
    ang = (pos / np.power(10000.0, bins / E)).astype(np.float32)
    return np.stack([np.sin(ang), np.cos(ang)], axis=2).reshape(n, E).astype(np.float32)


def _cat_heads(w):
    # [H, E, KD] -> [E, H*KD]
    return np.transpose(np.asarray(w, np.float32), (1, 0, 2)).reshape(E, HKD)


def _host_inputs(src, tgt, fp, src_mask, tgt_mask, params):
    f32 = np.float32
    src = np.asarray(src)
    tgt = np.asarray(tgt)
    fp = np.asarray(fp)
    pe = _pos_table(L)
    scale = math.sqrt(E)

    x0 = np.asarray(params["src_emb"], f32)[src] * scale + pe[None, :, :]
    xmm0 = np.asarray(params["fp_emb"], f32)[fp] * scale + pe[None, :, :]
    y0 = np.asarray(params["tgt_emb"], f32)[tgt] * scale + pe[None, :, :]

    sc = 1.0 / math.sqrt(KD)
    encw = np.zeros((NB, 4, E, HKD), f32)
    encw1 = np.zeros((NB, E, HD), f32)
    encw2 = np.zeros((NB, HD, E), f32)
    encc1b = np.zeros((NB, HD), f32)
    for l, pl in enumerate(params["enc"]):
        g1 = np.asarray(pl["g1"], f32)
        g2 = np.asarray(pl["g2"], f32)
        encw[l, 0] = _cat_heads(pl["Wq"]) * g1[:, None] * sc
        encw[l, 1] = _cat_heads(pl["Wk"]) * g1[:, None]
        encw[l, 2] = _cat_heads(pl["Wv"]) * g1[:, None]
        encw[l, 3] = np.asarray(pl["Wo"], f32)
        encw1[l] = np.asarray(pl["W1"], f32) * g2[:, None]
        encw2[l] = np.asarray(pl["W2"], f32)
        encc1b[l] = np.asarray(pl["c1b"], f32)

    eng = np.asarray(params["enc_norm_g"], f32)
    decw = np.zeros((NB, 8, E, HKD), f32)
    decw1 = np.zeros((NB, E, HD), f32)
    decw2 = np.zeros((NB, HD, E), f32)
    decc1b = np.zeros((NB, HD), f32)
    for l, pl in enumerate(params["dec"]):
        g1 = np.asarray(pl["g1"], f32)
        g2 = np.asarray(pl["g2"], f32)
        g3 = np.asarray(pl["g3"], f32)
        decw[l, 0] = _cat_heads(pl["Wq1"]) * g1[:, None] * sc
        decw[l, 1] = _cat_heads(pl["Wk1"]) * g1[:, None]
        decw[l, 2] = _cat_heads(pl["Wv1"]) * g1[:, None]
        decw[l, 3] = np.asarray(pl["Wo1"], f32)
        decw[l, 4] = _cat_heads(pl["Wq2"]) * g2[:, None] * sc
        decw[l, 5] = _cat_heads(pl["Wk2"]) * eng[:, None]
        decw[l, 6] = _cat_heads(pl["Wv2"]) * eng[:, None]
        decw[l, 7] = np.asarray(pl["Wo2"], f32)
        decw1[l] = np.asarray(pl["W1"], f32) * g3[:, None]
        decw2[l] = np.asarray(pl["W2"], f32)
        decc1b[l] = np.asarray(pl["c1b"], f32)

    dng = np.asarray(params["dec_norm_g"], f32)
    genw = np.asarray(params["gen_W"], f32) * dng[:, None]

    ident = np.eye(128, dtype=NPBF16)
    # transposed diagonal-block causal mask: mask[m, l] = 0 if m <= l else NEG
    mm_ = np.arange(128)[:, None]
    ll_ = np.arange(128)[None, :]
    cmask = np.where(mm_ <= ll_, 0.0, NEG).astype(f32)

    shared = dict(
        encw=encw.astype(NPBF16), encw1=encw1.astype(NPBF16),
        encw2=encw2.astype(NPBF16), encc1b=encc1b,
        decw=decw.astype(NPBF16), decw1=decw1.astype(NPBF16),
        decw2=decw2.astype(NPBF16), decc1b=decc1b,
        genw=genw.astype(NPBF16), ident=ident, cmask=cmask,
    )
    in_maps = []
    for i in range(NCORES):
        sl = slice(i * BL, (i + 1) * BL)
        m = dict(shared)
        m["x0"] = np.ascontiguousarray(x0[sl].reshape(T, E))
        m["xmm0"] = np.ascontiguousarray(xmm0[sl].reshape(T, E))
        m["y0"] = np.ascontiguousarray(y0[sl].reshape(T, E))
        in_maps.append(m)
    return in_maps


_NC_CACHE = {}


def _get_nc(n_enc=NB, n_dec=NB, out_what="logits"):
    key = (n_enc, n_dec, out_what)
    if key not in _NC_CACHE:
        _NC_CACHE[key] = build_nc(n_enc, n_dec, out_what)
    return _NC_CACHE[key]


def kernel(src, tgt, fp, src_mask, tgt_mask, params):
    nc = _get_nc()
    in_maps = _host_inputs(src, tgt, fp, src_mask, tgt_mask, params)
    res = bass_utils.run_bass_kernel_spmd(nc, in_maps, list(range(NCORES)))
    outs = [res.results[i]["out"].reshape(BL, L, VT) for i in range(NCORES)]
    return np.concatenate(outs, axis=0)
